# revision 4
# baseline (speedup 1.0000x reference)
"""GraphSAGE predictor on 8 Trainium2 NeuronCores (Bass/Tile).

Strategy (data-parallel over dst nodes, hint-aligned):
- Layer 1: each core computes 12500 of the 100000 dst rows. Feature rows are
  gathered with indirect DMA; the 10-neighbor sum is folded into the gather
  via the SDMA CCE add (compute_op=add into a memset tile). Self/neighbor
  terms go through PE transposes + matmuls, bias+relu fused on DVE/ACT.
- AllGather h1 across the 8 cores (padded shards of 12544 rows).
- Layer 2: same pattern over 5512 dst rows/core reading the gathered h1.
- AllGather h2, then the cosine-similarity head: per user, the item row is
  constant across the 10 sources, so b is gathered once per user; masked
  sources use an out-of-bounds sentinel + bounds_check so memset zeros
  survive, reproducing the zero-padding of rated_emb.
- All index composition (input_nodes[nbr1], padded-shard remapping,
  inverse_all composition) is host-side numpy on int32 index arrays only.
"""
import numpy as np

import concourse.bass as bass
import concourse.bacc as bacc
import concourse.tile as tile
import concourse.mybir as mybir
from concourse.bass_utils import run_bass_kernel_spmd
from concourse.masks import make_identity

NCORES = 8
N_ENT = 200000
F = 128
HID = 128
OUT = 64
N0 = 160000
N1 = 100000
N_ITEMS = 4096
N_RATED = 40000
N2 = N_ITEMS + N_RATED  # 44096
S = 10
B = 4096
R_TOT = 40960
N_MASKED = 960

L1_SH = N1 // NCORES          # 12500
L1_T = (L1_SH + 127) // 128   # 98
L1_PAD = L1_T * 128           # 12544
H1_FULL = L1_PAD * NCORES     # 100352

L2_SH = N2 // NCORES          # 5512
L2_T = (L2_SH + 127) // 128   # 44
L2_PAD = L2_T * 128           # 5632
H2_FULL = L2_PAD * NCORES     # 45056

HD_USERS = B // NCORES        # 512
HD_T = HD_USERS // 128        # 4

OOB = np.int32(1 << 20)

_compiled = None


def _build():
    dt = mybir.dt
    nc = bacc.Bacc("TRN2", target_bir_lowering=False, debug=False,
                   num_devices=NCORES)

    feat = nc.dram_tensor("feat", [N_ENT, F], dt.float32, kind="ExternalInput").ap()
    w1s = nc.dram_tensor("w1s", [F, HID], dt.float32, kind="ExternalInput").ap()
    w1n = nc.dram_tensor("w1n", [F, HID], dt.float32, kind="ExternalInput").ap()
    b1 = nc.dram_tensor("b1", [128, HID], dt.float32, kind="ExternalInput").ap()
    w2s = nc.dram_tensor("w2s", [HID, OUT], dt.float32, kind="ExternalInput").ap()
    w2n = nc.dram_tensor("w2n", [HID, OUT], dt.float32, kind="ExternalInput").ap()
    b2 = nc.dram_tensor("b2", [128, OUT], dt.float32, kind="ExternalInput").ap()

    l1n = nc.dram_tensor("l1n", [128, L1_T * S], dt.int32, kind="ExternalInput").ap()
    l1s = nc.dram_tensor("l1s", [128, L1_T], dt.int32, kind="ExternalInput").ap()
    l2n = nc.dram_tensor("l2n", [128, L2_T * S], dt.int32, kind="ExternalInput").ap()
    l2s = nc.dram_tensor("l2s", [128, L2_T], dt.int32, kind="ExternalInput").ap()
    hda = nc.dram_tensor("hda", [128, HD_T * S], dt.int32, kind="ExternalInput").ap()
    hdb = nc.dram_tensor("hdb", [128, HD_T], dt.int32, kind="ExternalInput").ap()

    pred = nc.dram_tensor("pred", [HD_T, 128], dt.float32, kind="ExternalOutput").ap()

    h1_loc = nc.dram_tensor("h1_loc", [L1_PAD, HID], dt.float32).ap()
    h1_full = nc.dram_tensor("h1_full", [H1_FULL, HID], dt.float32).ap()
    h2_loc = nc.dram_tensor("h2_loc", [L2_PAD, OUT], dt.float32).ap()
    h2_full = nc.dram_tensor("h2_full", [H2_FULL, OUT], dt.float32).ap()

    groups = [list(range(NCORES))]

    with tile.TileContext(nc) as tc:
        with (
            tc.tile_pool(name="const", bufs=1) as cpool,
            tc.tile_pool(name="work", bufs=4) as wpool,
            tc.tile_pool(name="psum", bufs=2, space="PSUM") as ppool,
        ):
            ident = cpool.tile([128, 128], dt.float32)
            make_identity(nc, ident[:])
            w1s_t = cpool.tile([F, HID], dt.float32)
            nc.sync.dma_start(w1s_t[:], w1s)
            w1n_t = cpool.tile([F, HID], dt.float32)
            nc.sync.dma_start(w1n_t[:], w1n)
            b1_t = cpool.tile([128, HID], dt.float32)
            nc.sync.dma_start(b1_t[:], b1)
            w2s_t = cpool.tile([HID, OUT], dt.float32)
            nc.sync.dma_start(w2s_t[:], w2s)
            w2n_t = cpool.tile([HID, OUT], dt.float32)
            nc.sync.dma_start(w2n_t[:], w2n)
            b2_t = cpool.tile([128, OUT], dt.float32)
            nc.sync.dma_start(b2_t[:], b2)

            l1n_t = cpool.tile([128, L1_T * S], dt.int32)
            nc.sync.dma_start(l1n_t[:], l1n)
            l1s_t = cpool.tile([128, L1_T], dt.int32)
            nc.sync.dma_start(l1s_t[:], l1s)
            l2n_t = cpool.tile([128, L2_T * S], dt.int32)
            nc.sync.dma_start(l2n_t[:], l2n)
            l2s_t = cpool.tile([128, L2_T], dt.int32)
            nc.sync.dma_start(l2s_t[:], l2s)
            hda_t = cpool.tile([128, HD_T * S], dt.int32)
            nc.sync.dma_start(hda_t[:], hda)
            hdb_t = cpool.tile([128, HD_T], dt.int32)
            nc.sync.dma_start(hdb_t[:], hdb)

            h1v = h1_loc.rearrange("(t p) f -> t p f", p=128)
            h2v = h2_loc.rearrange("(t p) f -> t p f", p=128)

            def sage_tile(t, nidx, sidx, src, wself, wnbr, bias_t, d_out, relu,
                          out_view):
                hnb = wpool.tile([128, F], dt.float32, tag="hnb", name="hnb")
                nc.vector.memset(hnb[:], 0.0)
                for j in range(S):
                    c = t * S + j
                    nc.gpsimd.indirect_dma_start(
                        out=hnb[:], out_offset=None, in_=src,
                        in_offset=bass.IndirectOffsetOnAxis(ap=nidx[:, c:c + 1], axis=0),
                        compute_op=mybir.AluOpType.add,
                    )
                hself = wpool.tile([128, F], dt.float32, tag="hself", name="hself")
                nc.gpsimd.indirect_dma_start(
                    out=hself[:], out_offset=None, in_=src,
                    in_offset=bass.IndirectOffsetOnAxis(ap=sidx[:, t:t + 1], axis=0),
                )
                pnb = ppool.tile([128, 128], dt.float32, tag="pnb", name="pnb")
                nc.tensor.transpose(pnb[:], hnb[:], ident[:])
                psf = ppool.tile([128, 128], dt.float32, tag="psf", name="psf")
                nc.tensor.transpose(psf[:], hself[:], ident[:])
                nbT = wpool.tile([128, 128], dt.float32, tag="nbT", name="nbT")
                nc.vector.tensor_copy(nbT[:], pnb[:])
                sfT = wpool.tile([128, 128], dt.float32, tag="sfT", name="sfT")
                nc.vector.tensor_copy(sfT[:], psf[:])
                pout = ppool.tile([128, d_out], dt.float32, tag="pout", name="pout")
                nc.tensor.matmul(pout[:], lhsT=sfT[:], rhs=wself[:], start=True, stop=False)
                nc.tensor.matmul(pout[:], lhsT=nbT[:], rhs=wnbr[:], start=False, stop=True)
                hout = wpool.tile([128, d_out], dt.float32, tag="hout", name="hout")
                nc.vector.tensor_tensor(
                    out=hout[:], in0=pout[:], in1=bias_t[:, :d_out],
                    op=mybir.AluOpType.add,
                )
                if relu:
                    nc.scalar.activation(hout[:], hout[:], mybir.ActivationFunctionType.Relu)
                nc.sync.dma_start(out_view[t], hout[:])

            for t in range(L1_T):
                sage_tile(t, l1n_t, l1s_t, feat, w1s_t, w1n_t, b1_t, HID, True, h1v)

            nc.gpsimd.collective_compute(
                "AllGather", mybir.AluOpType.bypass, replica_groups=groups,
                ins=[h1_loc.opt()], outs=[h1_full.opt()],
            )

            for t in range(L2_T):
                sage_tile(t, l2n_t, l2s_t, h1_full, w2s_t, w2n_t, b2_t, OUT, False, h2v)

            nc.gpsimd.collective_compute(
                "AllGather", mybir.AluOpType.bypass, replica_groups=groups,
                ins=[h2_loc.opt()], outs=[h2_full.opt()],
            )

            for t in range(HD_T):
                at = wpool.tile([128, S * OUT], dt.float32, tag="at", name="at")
                nc.vector.memset(at[:], 0.0)
                for j in range(S):
                    c = t * S + j
                    nc.gpsimd.indirect_dma_start(
                        out=at[:, j * OUT:(j + 1) * OUT], out_offset=None, in_=h2_full,
                        in_offset=bass.IndirectOffsetOnAxis(ap=hda_t[:, c:c + 1], axis=0),
                        bounds_check=H2_FULL - 1, oob_is_err=False,
                    )
                bt = wpool.tile([128, OUT], dt.float32, tag="bt", name="bt")
                nc.gpsimd.indirect_dma_start(
                    out=bt[:], out_offset=None, in_=h2_full,
                    in_offset=bass.IndirectOffsetOnAxis(ap=hdb_t[:, t:t + 1], axis=0),
                )
                ab = wpool.tile([128, S * OUT], dt.float32, tag="ab", name="ab")
                nc.vector.tensor_tensor(
                    out=ab[:].rearrange("p (s d) -> p s d", d=OUT),
                    in0=at[:].rearrange("p (s d) -> p s d", d=OUT),
                    in1=bt[:].unsqueeze(1).to_broadcast([128, S, OUT]),
                    op=mybir.AluOpType.mult,
                )
                dots = wpool.tile([128, S], dt.float32, tag="dots", name="dots")
                nc.vector.tensor_reduce(
                    out=dots[:], in_=ab[:].rearrange("p (s d) -> p s d", d=OUT),
                    axis=mybir.AxisListType.X, op=mybir.AluOpType.add,
                )
                nc.vector.tensor_tensor(out=ab[:], in0=at[:], in1=at[:], op=mybir.AluOpType.mult)
                na2 = wpool.tile([128, S], dt.float32, tag="na2", name="na2")
                nc.vector.tensor_reduce(
                    out=na2[:], in_=ab[:].rearrange("p (s d) -> p s d", d=OUT),
                    axis=mybir.AxisListType.X, op=mybir.AluOpType.add,
                )
                bb = wpool.tile([128, OUT], dt.float32, tag="bb", name="bb")
                nc.vector.tensor_tensor(out=bb[:], in0=bt[:], in1=bt[:], op=mybir.AluOpType.mult)
                nb2 = wpool.tile([128, 1], dt.float32, tag="nb2", name="nb2")
                nc.vector.tensor_reduce(
                    out=nb2[:], in_=bb[:], axis=mybir.AxisListType.X, op=mybir.AluOpType.add,
                )
                na = wpool.tile([128, S], dt.float32, tag="na", name="na")
                nc.scalar.activation(na[:], na2[:], mybir.ActivationFunctionType.Sqrt)
                nc.vector.tensor_scalar_max(na[:], na[:], 1e-6)
                nb = wpool.tile([128, 1], dt.float32, tag="nb", name="nb")
                nc.scalar.activation(nb[:], nb2[:], mybir.ActivationFunctionType.Sqrt)
                nc.vector.tensor_scalar_max(nb[:], nb[:], 1e-6)
                den = wpool.tile([128, S], dt.float32, tag="den", name="den")
                nc.vector.tensor_tensor(
                    out=den[:], in0=na[:], in1=nb[:].to_broadcast([128, S]),
                    op=mybir.AluOpType.mult,
                )
                rden = wpool.tile([128, S], dt.float32, tag="rden", name="rden")
                nc.vector.reciprocal(rden[:], den[:])
                sim = wpool.tile([128, S], dt.float32, tag="sim", name="sim")
                nc.vector.tensor_tensor(out=sim[:], in0=dots[:], in1=rden[:], op=mybir.AluOpType.mult)
                pr = wpool.tile([128, 1], dt.float32, tag="pr", name="pr")
                nc.vector.tensor_reduce(
                    out=pr[:], in_=sim[:], axis=mybir.AxisListType.X, op=mybir.AluOpType.add,
                )
                nc.sync.dma_start(pred[t], pr[:].rearrange("p o -> (p o)"))

    nc.compile()
    return nc


def _pad_map_l1(g):
    # global h1 row -> padded position in the allgathered h1
    return (g // L1_SH) * L1_PAD + (g % L1_SH)


def _pad_map_l2(q):
    return (q // L2_SH) * L2_PAD + (q % L2_SH)


def _tileize(a, ncols):
    """[T*128 rows, ncols] -> [128, T*ncols] partition-major tile layout."""
    T = a.shape[0] // 128
    return np.ascontiguousarray(
        a.reshape(T, 128, ncols).transpose(1, 0, 2).reshape(128, T * ncols)
    ).astype(np.int32)


def kernel(features, Wself1, Wnbr1, b1, Wself2, Wnbr2, b2,
           input_nodes, nbr1, nbr2, inverse_all, source, item_rep_idx,
           n_items, n_masked):
    global _compiled
    if _compiled is None:
        _compiled = _build()
    nc = _compiled

    features = np.asarray(features, dtype=np.float32)
    input_nodes = np.asarray(input_nodes, dtype=np.int64)
    nbr1 = np.asarray(nbr1, dtype=np.int64)
    nbr2 = np.asarray(nbr2, dtype=np.int64)
    inverse_all = np.asarray(inverse_all, dtype=np.int64)
    source = np.asarray(source, dtype=np.int64)
    item_rep_idx = np.asarray(item_rep_idx, dtype=np.int64)

    scale = np.float32(1.0 / S)
    w1n_s = (np.asarray(Wnbr1, np.float32) * scale).astype(np.float32)
    w2n_s = (np.asarray(Wnbr2, np.float32) * scale).astype(np.float32)

    common = {
        "feat": features,
        "w1s": np.asarray(Wself1, np.float32),
        "w1n": w1n_s,
        "b1": np.tile(np.asarray(b1, np.float32).reshape(1, HID), (128, 1)),
        "w2s": np.asarray(Wself2, np.float32),
        "w2n": w2n_s,
        "b2": np.tile(np.asarray(b2, np.float32).reshape(1, OUT), (128, 1)),
    }

    in_maps = []
    for k in range(NCORES):
        # ---- layer 1 indices (into features) ----
        d0 = k * L1_SH
        d = np.arange(L1_PAD) + d0
        real = d < d0 + L1_SH
        d_c = np.where(real, d, d0)  # clamp padding to a real row
        l1n_idx = input_nodes[nbr1[d_c]]           # [L1_PAD, S]
        l1s_idx = input_nodes[d_c][:, None]        # [L1_PAD, 1]

        # ---- layer 2 indices (into padded h1) ----
        g0 = k * L2_SH
        g = np.arange(L2_PAD) + g0
        realg = g < g0 + L2_SH
        g_c = np.where(realg, g, g0)
        l2n_idx = _pad_map_l1(nbr1_safe := nbr2[g_c])  # [L2_PAD, S]
        l2s_idx = _pad_map_l1(g_c)[:, None]

        # ---- head indices (into padded h2) ----
        u0 = k * HD_USERS
        u = np.arange(HD_USERS) + u0
        src = source.reshape(B, S)[u]               # [512, S]
        masked = src < n_masked
        a_q = n_items + (src - n_masked)            # all_emb row if unmasked
        a_idx = np.where(masked, OOB, _pad_map_l2(inverse_all[np.where(masked, 0, a_q)]))
        b_idx = _pad_map_l2(inverse_all[item_rep_idx.reshape(B, S)[u, 0]])[:, None]

        in_maps.append({
            **common,
            "l1n": _tileize(l1n_idx, S),
            "l1s": _tileize(l1s_idx, 1),
            "l2n": _tileize(l2n_idx, S),
            "l2s": _tileize(l2s_idx, 1),
            "hda": _tileize(a_idx, S),
            "hdb": _tileize(b_idx, 1),
        })

    res = run_bass_kernel_spmd(nc, in_maps, core_ids=list(range(NCORES)))
    pred = np.concatenate(
        [res.results[k]["pred"].reshape(HD_T, 128).reshape(-1) for k in range(NCORES)]
    )
    return pred.astype(np.float32)


# revision 5
# speedup vs baseline: 1.3503x; 1.3503x over previous
"""GraphSAGE predictor on 8 Trainium2 NeuronCores (Bass/Tile).

Strategy (data-parallel over dst nodes, hint-aligned):
- Layer 1: each core computes 12500 of the 100000 dst rows. Feature rows are
  gathered with indirect DMA; the 10-neighbor sum is folded into the gather
  via the SDMA CCE add (compute_op=add into a memset tile). Self/neighbor
  terms go through PE transposes + matmuls, bias+relu fused on DVE/ACT.
- AllGather h1 across the 8 cores (padded shards of 12544 rows).
- Layer 2: same pattern over 5512 dst rows/core reading the gathered h1.
- AllGather h2, then the cosine-similarity head: per user, the item row is
  constant across the 10 sources, so b is gathered once per user; masked
  sources use an out-of-bounds sentinel + bounds_check so memset zeros
  survive, reproducing the zero-padding of rated_emb.
- All index composition (input_nodes[nbr1], padded-shard remapping,
  inverse_all composition) is host-side numpy on int32 index arrays only.
"""
import numpy as np

import concourse.bass as bass
import concourse.bacc as bacc
import concourse.tile as tile
import concourse.mybir as mybir
from concourse.bass_utils import run_bass_kernel_spmd
from concourse.masks import make_identity

NCORES = 8
N_ENT = 200000
F = 128
HID = 128
OUT = 64
N0 = 160000
N1 = 100000
N_ITEMS = 4096
N_RATED = 40000
N2 = N_ITEMS + N_RATED  # 44096
S = 10
B = 4096
R_TOT = 40960
N_MASKED = 960

L1_SH = N1 // NCORES          # 12500
L1_T = (L1_SH + 127) // 128   # 98
L1_PAD = L1_T * 128           # 12544
H1_FULL = L1_PAD * NCORES     # 100352

L2_SH = N2 // NCORES          # 5512
L2_T = (L2_SH + 127) // 128   # 44
L2_PAD = L2_T * 128           # 5632
H2_FULL = L2_PAD * NCORES     # 45056

HD_USERS = B // NCORES        # 512
HD_T = HD_USERS // 128        # 4

OOB = np.int32(1 << 20)

_compiled = None


def _build():
    dt = mybir.dt
    nc = bacc.Bacc("TRN2", target_bir_lowering=False, debug=False,
                   num_devices=NCORES)

    feat = nc.dram_tensor("feat", [N_ENT, F], dt.float32, kind="ExternalInput").ap()
    w1s = nc.dram_tensor("w1s", [F, HID], dt.float32, kind="ExternalInput").ap()
    w1n = nc.dram_tensor("w1n", [F, HID], dt.float32, kind="ExternalInput").ap()
    b1 = nc.dram_tensor("b1", [128, HID], dt.float32, kind="ExternalInput").ap()
    w2s = nc.dram_tensor("w2s", [HID, OUT], dt.float32, kind="ExternalInput").ap()
    w2n = nc.dram_tensor("w2n", [HID, OUT], dt.float32, kind="ExternalInput").ap()
    b2 = nc.dram_tensor("b2", [128, OUT], dt.float32, kind="ExternalInput").ap()

    l1n = nc.dram_tensor("l1n", [128, L1_T * S], dt.int32, kind="ExternalInput").ap()
    l1s = nc.dram_tensor("l1s", [128, L1_T], dt.int32, kind="ExternalInput").ap()
    l2n = nc.dram_tensor("l2n", [128, L2_T * S], dt.int32, kind="ExternalInput").ap()
    l2s = nc.dram_tensor("l2s", [128, L2_T], dt.int32, kind="ExternalInput").ap()
    hda = nc.dram_tensor("hda", [128, HD_T * S], dt.int32, kind="ExternalInput").ap()
    hdb = nc.dram_tensor("hdb", [128, HD_T], dt.int32, kind="ExternalInput").ap()

    pred = nc.dram_tensor("pred", [HD_T, 128], dt.float32, kind="ExternalOutput").ap()

    h1_loc = nc.dram_tensor("h1_loc", [L1_PAD, HID], dt.float32).ap()
    h1_full = nc.dram_tensor("h1_full", [H1_FULL, HID], dt.float32).ap()
    h2_loc = nc.dram_tensor("h2_loc", [L2_PAD, OUT], dt.float32).ap()
    h2_full = nc.dram_tensor("h2_full", [H2_FULL, OUT], dt.float32).ap()

    groups = [list(range(NCORES))]

    with tile.TileContext(nc) as tc:
        with (
            tc.tile_pool(name="const", bufs=1) as cpool,
            tc.tile_pool(name="work", bufs=4) as wpool,
            tc.tile_pool(name="psum", bufs=2, space="PSUM") as ppool,
        ):
            ident = cpool.tile([128, 128], dt.float32)
            make_identity(nc, ident[:])
            w1s_t = cpool.tile([F, HID], dt.float32)
            nc.sync.dma_start(w1s_t[:], w1s)
            w1n_t = cpool.tile([F, HID], dt.float32)
            nc.sync.dma_start(w1n_t[:], w1n)
            b1_t = cpool.tile([128, HID], dt.float32)
            nc.sync.dma_start(b1_t[:], b1)
            w2s_t = cpool.tile([HID, OUT], dt.float32)
            nc.sync.dma_start(w2s_t[:], w2s)
            w2n_t = cpool.tile([HID, OUT], dt.float32)
            nc.sync.dma_start(w2n_t[:], w2n)
            b2_t = cpool.tile([128, OUT], dt.float32)
            nc.sync.dma_start(b2_t[:], b2)

            l1n_t = cpool.tile([128, L1_T * S], dt.int32)
            nc.sync.dma_start(l1n_t[:], l1n)
            l1s_t = cpool.tile([128, L1_T], dt.int32)
            nc.sync.dma_start(l1s_t[:], l1s)
            l2n_t = cpool.tile([128, L2_T * S], dt.int32)
            nc.sync.dma_start(l2n_t[:], l2n)
            l2s_t = cpool.tile([128, L2_T], dt.int32)
            nc.sync.dma_start(l2s_t[:], l2s)
            hda_t = cpool.tile([128, HD_T * S], dt.int32)
            nc.sync.dma_start(hda_t[:], hda)
            hdb_t = cpool.tile([128, HD_T], dt.int32)
            nc.sync.dma_start(hdb_t[:], hdb)

            h1v = h1_loc.rearrange("(t p) f -> t p f", p=128)
            h2v = h2_loc.rearrange("(t p) f -> t p f", p=128)

            def sage_tile(t, nidx, sidx, src, wself, wnbr, bias_t, d_out, relu,
                          out_view):
                wide = wpool.tile([128, S * F], dt.float32, tag="wide", name="wide")
                for j in range(S):
                    c = t * S + j
                    nc.gpsimd.indirect_dma_start(
                        out=wide[:, j * F:(j + 1) * F], out_offset=None, in_=src,
                        in_offset=bass.IndirectOffsetOnAxis(ap=nidx[:, c:c + 1], axis=0),
                    )
                hself = wpool.tile([128, F], dt.float32, tag="hself", name="hself")
                nc.gpsimd.indirect_dma_start(
                    out=hself[:], out_offset=None, in_=src,
                    in_offset=bass.IndirectOffsetOnAxis(ap=sidx[:, t:t + 1], axis=0),
                )
                pnb = ppool.tile([128, 128], dt.float32, tag="pnb", name="pnb")
                for j in range(S):
                    nc.tensor.matmul(pnb[:], lhsT=wide[:, j * F:(j + 1) * F],
                                     rhs=ident[:], is_transpose=True,
                                     start=(j == 0), stop=(j == S - 1))
                psf = ppool.tile([128, 128], dt.float32, tag="psf", name="psf")
                nc.tensor.transpose(psf[:], hself[:], ident[:])
                nbT = wpool.tile([128, 128], dt.float32, tag="nbT", name="nbT")
                nc.vector.tensor_copy(nbT[:], pnb[:])
                sfT = wpool.tile([128, 128], dt.float32, tag="sfT", name="sfT")
                nc.vector.tensor_copy(sfT[:], psf[:])
                pout = ppool.tile([128, d_out], dt.float32, tag="pout", name="pout")
                nc.tensor.matmul(pout[:], lhsT=sfT[:], rhs=wself[:], start=True, stop=False)
                nc.tensor.matmul(pout[:], lhsT=nbT[:], rhs=wnbr[:], start=False, stop=True)
                hout = wpool.tile([128, d_out], dt.float32, tag="hout", name="hout")
                nc.vector.tensor_tensor(
                    out=hout[:], in0=pout[:], in1=bias_t[:, :d_out],
                    op=mybir.AluOpType.add,
                )
                if relu:
                    nc.scalar.activation(hout[:], hout[:], mybir.ActivationFunctionType.Relu)
                nc.sync.dma_start(out_view[t], hout[:])

            for t in range(L1_T):
                sage_tile(t, l1n_t, l1s_t, feat, w1s_t, w1n_t, b1_t, HID, True, h1v)

            nc.gpsimd.collective_compute(
                "AllGather", mybir.AluOpType.bypass, replica_groups=groups,
                ins=[h1_loc.opt()], outs=[h1_full.opt()],
            )

            for t in range(L2_T):
                sage_tile(t, l2n_t, l2s_t, h1_full, w2s_t, w2n_t, b2_t, OUT, False, h2v)

            nc.gpsimd.collective_compute(
                "AllGather", mybir.AluOpType.bypass, replica_groups=groups,
                ins=[h2_loc.opt()], outs=[h2_full.opt()],
            )

            for t in range(HD_T):
                at = wpool.tile([128, S * OUT], dt.float32, tag="at", name="at")
                nc.vector.memset(at[:], 0.0)
                for j in range(S):
                    c = t * S + j
                    nc.gpsimd.indirect_dma_start(
                        out=at[:, j * OUT:(j + 1) * OUT], out_offset=None, in_=h2_full,
                        in_offset=bass.IndirectOffsetOnAxis(ap=hda_t[:, c:c + 1], axis=0),
                        bounds_check=H2_FULL - 1, oob_is_err=False,
                    )
                bt = wpool.tile([128, OUT], dt.float32, tag="bt", name="bt")
                nc.gpsimd.indirect_dma_start(
                    out=bt[:], out_offset=None, in_=h2_full,
                    in_offset=bass.IndirectOffsetOnAxis(ap=hdb_t[:, t:t + 1], axis=0),
                )
                ab = wpool.tile([128, S * OUT], dt.float32, tag="ab", name="ab")
                nc.vector.tensor_tensor(
                    out=ab[:].rearrange("p (s d) -> p s d", d=OUT),
                    in0=at[:].rearrange("p (s d) -> p s d", d=OUT),
                    in1=bt[:].unsqueeze(1).to_broadcast([128, S, OUT]),
                    op=mybir.AluOpType.mult,
                )
                dots = wpool.tile([128, S], dt.float32, tag="dots", name="dots")
                nc.vector.tensor_reduce(
                    out=dots[:], in_=ab[:].rearrange("p (s d) -> p s d", d=OUT),
                    axis=mybir.AxisListType.X, op=mybir.AluOpType.add,
                )
                nc.vector.tensor_tensor(out=ab[:], in0=at[:], in1=at[:], op=mybir.AluOpType.mult)
                na2 = wpool.tile([128, S], dt.float32, tag="na2", name="na2")
                nc.vector.tensor_reduce(
                    out=na2[:], in_=ab[:].rearrange("p (s d) -> p s d", d=OUT),
                    axis=mybir.AxisListType.X, op=mybir.AluOpType.add,
                )
                bb = wpool.tile([128, OUT], dt.float32, tag="bb", name="bb")
                nc.vector.tensor_tensor(out=bb[:], in0=bt[:], in1=bt[:], op=mybir.AluOpType.mult)
                nb2 = wpool.tile([128, 1], dt.float32, tag="nb2", name="nb2")
                nc.vector.tensor_reduce(
                    out=nb2[:], in_=bb[:], axis=mybir.AxisListType.X, op=mybir.AluOpType.add,
                )
                na = wpool.tile([128, S], dt.float32, tag="na", name="na")
                nc.scalar.activation(na[:], na2[:], mybir.ActivationFunctionType.Sqrt)
                nc.vector.tensor_scalar_max(na[:], na[:], 1e-6)
                nb = wpool.tile([128, 1], dt.float32, tag="nb", name="nb")
                nc.scalar.activation(nb[:], nb2[:], mybir.ActivationFunctionType.Sqrt)
                nc.vector.tensor_scalar_max(nb[:], nb[:], 1e-6)
                den = wpool.tile([128, S], dt.float32, tag="den", name="den")
                nc.vector.tensor_tensor(
                    out=den[:], in0=na[:], in1=nb[:].to_broadcast([128, S]),
                    op=mybir.AluOpType.mult,
                )
                rden = wpool.tile([128, S], dt.float32, tag="rden", name="rden")
                nc.vector.reciprocal(rden[:], den[:])
                sim = wpool.tile([128, S], dt.float32, tag="sim", name="sim")
                nc.vector.tensor_tensor(out=sim[:], in0=dots[:], in1=rden[:], op=mybir.AluOpType.mult)
                pr = wpool.tile([128, 1], dt.float32, tag="pr", name="pr")
                nc.vector.tensor_reduce(
                    out=pr[:], in_=sim[:], axis=mybir.AxisListType.X, op=mybir.AluOpType.add,
                )
                nc.sync.dma_start(pred[t], pr[:].rearrange("p o -> (p o)"))

    nc.compile()
    return nc


def _pad_map_l1(g):
    # global h1 row -> padded position in the allgathered h1
    return (g // L1_SH) * L1_PAD + (g % L1_SH)


def _pad_map_l2(q):
    return (q // L2_SH) * L2_PAD + (q % L2_SH)


def _tileize(a, ncols):
    """[T*128 rows, ncols] -> [128, T*ncols] partition-major tile layout."""
    T = a.shape[0] // 128
    return np.ascontiguousarray(
        a.reshape(T, 128, ncols).transpose(1, 0, 2).reshape(128, T * ncols)
    ).astype(np.int32)


def kernel(features, Wself1, Wnbr1, b1, Wself2, Wnbr2, b2,
           input_nodes, nbr1, nbr2, inverse_all, source, item_rep_idx,
           n_items, n_masked):
    global _compiled
    if _compiled is None:
        _compiled = _build()
    nc = _compiled

    features = np.asarray(features, dtype=np.float32)
    input_nodes = np.asarray(input_nodes, dtype=np.int64)
    nbr1 = np.asarray(nbr1, dtype=np.int64)
    nbr2 = np.asarray(nbr2, dtype=np.int64)
    inverse_all = np.asarray(inverse_all, dtype=np.int64)
    source = np.asarray(source, dtype=np.int64)
    item_rep_idx = np.asarray(item_rep_idx, dtype=np.int64)

    scale = np.float32(1.0 / S)
    w1n_s = (np.asarray(Wnbr1, np.float32) * scale).astype(np.float32)
    w2n_s = (np.asarray(Wnbr2, np.float32) * scale).astype(np.float32)

    common = {
        "feat": features,
        "w1s": np.asarray(Wself1, np.float32),
        "w1n": w1n_s,
        "b1": np.tile(np.asarray(b1, np.float32).reshape(1, HID), (128, 1)),
        "w2s": np.asarray(Wself2, np.float32),
        "w2n": w2n_s,
        "b2": np.tile(np.asarray(b2, np.float32).reshape(1, OUT), (128, 1)),
    }

    in_maps = []
    for k in range(NCORES):
        # ---- layer 1 indices (into features) ----
        d0 = k * L1_SH
        d = np.arange(L1_PAD) + d0
        real = d < d0 + L1_SH
        d_c = np.where(real, d, d0)  # clamp padding to a real row
        l1n_idx = input_nodes[nbr1[d_c]]           # [L1_PAD, S]
        l1s_idx = input_nodes[d_c][:, None]        # [L1_PAD, 1]

        # ---- layer 2 indices (into padded h1) ----
        g0 = k * L2_SH
        g = np.arange(L2_PAD) + g0
        realg = g < g0 + L2_SH
        g_c = np.where(realg, g, g0)
        l2n_idx = _pad_map_l1(nbr1_safe := nbr2[g_c])  # [L2_PAD, S]
        l2s_idx = _pad_map_l1(g_c)[:, None]

        # ---- head indices (into padded h2) ----
        u0 = k * HD_USERS
        u = np.arange(HD_USERS) + u0
        src = source.reshape(B, S)[u]               # [512, S]
        masked = src < n_masked
        a_q = n_items + (src - n_masked)            # all_emb row if unmasked
        a_idx = np.where(masked, OOB, _pad_map_l2(inverse_all[np.where(masked, 0, a_q)]))
        b_idx = _pad_map_l2(inverse_all[item_rep_idx.reshape(B, S)[u, 0]])[:, None]

        in_maps.append({
            **common,
            "l1n": _tileize(l1n_idx, S),
            "l1s": _tileize(l1s_idx, 1),
            "l2n": _tileize(l2n_idx, S),
            "l2s": _tileize(l2s_idx, 1),
            "hda": _tileize(a_idx, S),
            "hdb": _tileize(b_idx, 1),
        })

    res = run_bass_kernel_spmd(nc, in_maps, core_ids=list(range(NCORES)))
    pred = np.concatenate(
        [res.results[k]["pred"].reshape(HD_T, 128).reshape(-1) for k in range(NCORES)]
    )
    return pred.astype(np.float32)


# revision 6
# speedup vs baseline: 1.4450x; 1.0701x over previous
"""GraphSAGE predictor on 8 Trainium2 NeuronCores (Bass/Tile).

Strategy (data-parallel over dst nodes, hint-aligned):
- Layer 1: each core computes 12500 of the 100000 dst rows. Feature rows are
  gathered with indirect DMA; the 10-neighbor sum is folded into the gather
  via the SDMA CCE add (compute_op=add into a memset tile). Self/neighbor
  terms go through PE transposes + matmuls, bias+relu fused on DVE/ACT.
- AllGather h1 across the 8 cores (padded shards of 12544 rows).
- Layer 2: same pattern over 5512 dst rows/core reading the gathered h1.
- AllGather h2, then the cosine-similarity head: per user, the item row is
  constant across the 10 sources, so b is gathered once per user; masked
  sources use an out-of-bounds sentinel + bounds_check so memset zeros
  survive, reproducing the zero-padding of rated_emb.
- All index composition (input_nodes[nbr1], padded-shard remapping,
  inverse_all composition) is host-side numpy on int32 index arrays only.
"""
import numpy as np

import concourse.bass as bass
import concourse.bacc as bacc
import concourse.tile as tile
import concourse.mybir as mybir
from concourse.bass_utils import run_bass_kernel_spmd
from concourse.masks import make_identity

NCORES = 8
N_ENT = 200000
F = 128
HID = 128
OUT = 64
N0 = 160000
N1 = 100000
N_ITEMS = 4096
N_RATED = 40000
N2 = N_ITEMS + N_RATED  # 44096
S = 10
B = 4096
R_TOT = 40960
N_MASKED = 960

L1_SH = N1 // NCORES          # 12500
L1_T = (L1_SH + 127) // 128   # 98
L1_PAD = L1_T * 128           # 12544
H1_FULL = L1_PAD * NCORES     # 100352
L1_CHUNKS = [25, 25, 24, 24]              # tiles per AllGather chunk
L1_CH_ROWS = [c * 128 for c in L1_CHUNKS]
L1_CH_R0 = [0, 3200, 6400, 9472]          # local row start of each chunk
L1_CH_BASE = [0, 25600, 51200, 75776]     # chunk base in gathered h1

L2_SH = N2 // NCORES          # 5512
L2_T = (L2_SH + 127) // 128   # 44
L2_PAD = L2_T * 128           # 5632
H2_FULL = L2_PAD * NCORES     # 45056
L2_CHUNKS = [22, 22]
L2_CH_ROWS = [c * 128 for c in L2_CHUNKS]
L2_CH_R0 = [0, 2816]
L2_CH_BASE = [0, 22528]

HD_USERS = B // NCORES        # 512
HD_T = HD_USERS // 128        # 4

OOB = np.int32(1 << 20)

_compiled = None


def _build():
    dt = mybir.dt
    nc = bacc.Bacc("TRN2", target_bir_lowering=False, debug=False,
                   num_devices=NCORES)

    feat = nc.dram_tensor("feat", [N_ENT, F], dt.float32, kind="ExternalInput").ap()
    w1s = nc.dram_tensor("w1s", [F, HID], dt.float32, kind="ExternalInput").ap()
    w1n = nc.dram_tensor("w1n", [F, HID], dt.float32, kind="ExternalInput").ap()
    b1 = nc.dram_tensor("b1", [128, HID], dt.float32, kind="ExternalInput").ap()
    w2s = nc.dram_tensor("w2s", [HID, OUT], dt.float32, kind="ExternalInput").ap()
    w2n = nc.dram_tensor("w2n", [HID, OUT], dt.float32, kind="ExternalInput").ap()
    b2 = nc.dram_tensor("b2", [128, OUT], dt.float32, kind="ExternalInput").ap()

    l1n = nc.dram_tensor("l1n", [128, L1_T * S], dt.int32, kind="ExternalInput").ap()
    l1s = nc.dram_tensor("l1s", [128, L1_T], dt.int32, kind="ExternalInput").ap()
    l2n = nc.dram_tensor("l2n", [128, L2_T * S], dt.int32, kind="ExternalInput").ap()
    l2s = nc.dram_tensor("l2s", [128, L2_T], dt.int32, kind="ExternalInput").ap()
    hda = nc.dram_tensor("hda", [128, HD_T * S], dt.int32, kind="ExternalInput").ap()
    hdb = nc.dram_tensor("hdb", [128, HD_T], dt.int32, kind="ExternalInput").ap()

    pred = nc.dram_tensor("pred", [HD_T, 128], dt.float32, kind="ExternalOutput").ap()

    h1_loc = nc.dram_tensor("h1_loc", [L1_PAD, HID], dt.float32).ap()
    h1_full = nc.dram_tensor("h1_full", [H1_FULL, HID], dt.float32).ap()
    h2_loc = nc.dram_tensor("h2_loc", [L2_PAD, OUT], dt.float32).ap()
    h2_full = nc.dram_tensor("h2_full", [H2_FULL, OUT], dt.float32).ap()

    groups = [list(range(NCORES))]

    with tile.TileContext(nc) as tc:
        with (
            tc.tile_pool(name="const", bufs=1) as cpool,
            tc.tile_pool(name="work", bufs=6) as wpool,
            tc.tile_pool(name="psum", bufs=3, space="PSUM") as ppool,
            tc.tile_pool(name="psumo", bufs=2, space="PSUM") as ppool_o,
        ):
            ident = cpool.tile([128, 128], dt.float32)
            make_identity(nc, ident[:])
            w1s_t = cpool.tile([F, HID], dt.float32)
            nc.sync.dma_start(w1s_t[:], w1s)
            w1n_t = cpool.tile([F, HID], dt.float32)
            nc.sync.dma_start(w1n_t[:], w1n)
            b1_t = cpool.tile([128, HID], dt.float32)
            nc.sync.dma_start(b1_t[:], b1)
            w2s_t = cpool.tile([HID, OUT], dt.float32)
            nc.sync.dma_start(w2s_t[:], w2s)
            w2n_t = cpool.tile([HID, OUT], dt.float32)
            nc.sync.dma_start(w2n_t[:], w2n)
            b2_t = cpool.tile([128, OUT], dt.float32)
            nc.sync.dma_start(b2_t[:], b2)

            l1n_t = cpool.tile([128, L1_T * S], dt.int32)
            nc.sync.dma_start(l1n_t[:], l1n)
            l1s_t = cpool.tile([128, L1_T], dt.int32)
            nc.sync.dma_start(l1s_t[:], l1s)
            l2n_t = cpool.tile([128, L2_T * S], dt.int32)
            nc.sync.dma_start(l2n_t[:], l2n)
            l2s_t = cpool.tile([128, L2_T], dt.int32)
            nc.sync.dma_start(l2s_t[:], l2s)
            hda_t = cpool.tile([128, HD_T * S], dt.int32)
            nc.sync.dma_start(hda_t[:], hda)
            hdb_t = cpool.tile([128, HD_T], dt.int32)
            nc.sync.dma_start(hdb_t[:], hdb)

            h1v = h1_loc.rearrange("(t p) f -> t p f", p=128)
            h2v = h2_loc.rearrange("(t p) f -> t p f", p=128)

            def sage_tile(t, nidx, sidx, src, wself, wnbr, bias_t, d_out, relu,
                          out_view):
                wide = wpool.tile([128, S * F], dt.float32, tag="wide", name="wide")
                for j in range(S):
                    c = t * S + j
                    nc.gpsimd.indirect_dma_start(
                        out=wide[:, j * F:(j + 1) * F], out_offset=None, in_=src,
                        in_offset=bass.IndirectOffsetOnAxis(ap=nidx[:, c:c + 1], axis=0),
                    )
                hself = wpool.tile([128, F], dt.float32, tag="hself", name="hself")
                nc.gpsimd.indirect_dma_start(
                    out=hself[:], out_offset=None, in_=src,
                    in_offset=bass.IndirectOffsetOnAxis(ap=sidx[:, t:t + 1], axis=0),
                )
                pnb = ppool.tile([128, 128], dt.float32, tag="pnb", name="pnb")
                for j in range(S):
                    nc.tensor.matmul(pnb[:], lhsT=wide[:, j * F:(j + 1) * F],
                                     rhs=ident[:], is_transpose=True,
                                     start=(j == 0), stop=(j == S - 1))
                psf = ppool.tile([128, 128], dt.float32, tag="psf", name="psf")
                nc.tensor.transpose(psf[:], hself[:], ident[:])
                nbT = wpool.tile([128, 128], dt.float32, tag="nbT", name="nbT")
                nc.vector.tensor_copy(nbT[:], pnb[:])
                sfT = wpool.tile([128, 128], dt.float32, tag="sfT", name="sfT")
                nc.vector.tensor_copy(sfT[:], psf[:])
                pout = ppool_o.tile([128, d_out], dt.float32, tag="pout", name="pout")
                nc.tensor.matmul(pout[:], lhsT=sfT[:], rhs=wself[:], start=True, stop=False)
                nc.tensor.matmul(pout[:], lhsT=nbT[:], rhs=wnbr[:], start=False, stop=True)
                hout = wpool.tile([128, d_out], dt.float32, tag="hout", name="hout")
                nc.vector.tensor_tensor(
                    out=hout[:], in0=pout[:], in1=bias_t[:, :d_out],
                    op=mybir.AluOpType.add,
                )
                if relu:
                    nc.scalar.activation(hout[:], hout[:], mybir.ActivationFunctionType.Relu)
                nc.sync.dma_start(out_view[t], hout[:])

            t = 0
            for c, ntiles in enumerate(L1_CHUNKS):
                for _ in range(ntiles):
                    sage_tile(t, l1n_t, l1s_t, feat, w1s_t, w1n_t, b1_t, HID, True, h1v)
                    t += 1
                r0, rows, base = L1_CH_R0[c], L1_CH_ROWS[c], L1_CH_BASE[c]
                nc.gpsimd.collective_compute(
                    "AllGather", mybir.AluOpType.bypass, replica_groups=groups,
                    ins=[h1_loc[r0:r0 + rows].opt()],
                    outs=[h1_full[base:base + NCORES * rows].opt()],
                )

            t = 0
            for c, ntiles in enumerate(L2_CHUNKS):
                for _ in range(ntiles):
                    sage_tile(t, l2n_t, l2s_t, h1_full, w2s_t, w2n_t, b2_t, OUT, False, h2v)
                    t += 1
                r0, rows, base = L2_CH_R0[c], L2_CH_ROWS[c], L2_CH_BASE[c]
                nc.gpsimd.collective_compute(
                    "AllGather", mybir.AluOpType.bypass, replica_groups=groups,
                    ins=[h2_loc[r0:r0 + rows].opt()],
                    outs=[h2_full[base:base + NCORES * rows].opt()],
                )

            for t in range(HD_T):
                at = wpool.tile([128, S * OUT], dt.float32, tag="at", name="at")
                nc.vector.memset(at[:], 0.0)
                for j in range(S):
                    c = t * S + j
                    nc.gpsimd.indirect_dma_start(
                        out=at[:, j * OUT:(j + 1) * OUT], out_offset=None, in_=h2_full,
                        in_offset=bass.IndirectOffsetOnAxis(ap=hda_t[:, c:c + 1], axis=0),
                        bounds_check=H2_FULL - 1, oob_is_err=False,
                    )
                bt = wpool.tile([128, OUT], dt.float32, tag="bt", name="bt")
                nc.gpsimd.indirect_dma_start(
                    out=bt[:], out_offset=None, in_=h2_full,
                    in_offset=bass.IndirectOffsetOnAxis(ap=hdb_t[:, t:t + 1], axis=0),
                )
                ab = wpool.tile([128, S * OUT], dt.float32, tag="ab", name="ab")
                nc.vector.tensor_tensor(
                    out=ab[:].rearrange("p (s d) -> p s d", d=OUT),
                    in0=at[:].rearrange("p (s d) -> p s d", d=OUT),
                    in1=bt[:].unsqueeze(1).to_broadcast([128, S, OUT]),
                    op=mybir.AluOpType.mult,
                )
                dots = wpool.tile([128, S], dt.float32, tag="dots", name="dots")
                nc.vector.tensor_reduce(
                    out=dots[:], in_=ab[:].rearrange("p (s d) -> p s d", d=OUT),
                    axis=mybir.AxisListType.X, op=mybir.AluOpType.add,
                )
                nc.vector.tensor_tensor(out=ab[:], in0=at[:], in1=at[:], op=mybir.AluOpType.mult)
                na2 = wpool.tile([128, S], dt.float32, tag="na2", name="na2")
                nc.vector.tensor_reduce(
                    out=na2[:], in_=ab[:].rearrange("p (s d) -> p s d", d=OUT),
                    axis=mybir.AxisListType.X, op=mybir.AluOpType.add,
                )
                bb = wpool.tile([128, OUT], dt.float32, tag="bb", name="bb")
                nc.vector.tensor_tensor(out=bb[:], in0=bt[:], in1=bt[:], op=mybir.AluOpType.mult)
                nb2 = wpool.tile([128, 1], dt.float32, tag="nb2", name="nb2")
                nc.vector.tensor_reduce(
                    out=nb2[:], in_=bb[:], axis=mybir.AxisListType.X, op=mybir.AluOpType.add,
                )
                na = wpool.tile([128, S], dt.float32, tag="na", name="na")
                nc.scalar.activation(na[:], na2[:], mybir.ActivationFunctionType.Sqrt)
                nc.vector.tensor_scalar_max(na[:], na[:], 1e-6)
                nb = wpool.tile([128, 1], dt.float32, tag="nb", name="nb")
                nc.scalar.activation(nb[:], nb2[:], mybir.ActivationFunctionType.Sqrt)
                nc.vector.tensor_scalar_max(nb[:], nb[:], 1e-6)
                den = wpool.tile([128, S], dt.float32, tag="den", name="den")
                nc.vector.tensor_tensor(
                    out=den[:], in0=na[:], in1=nb[:].to_broadcast([128, S]),
                    op=mybir.AluOpType.mult,
                )
                rden = wpool.tile([128, S], dt.float32, tag="rden", name="rden")
                nc.vector.reciprocal(rden[:], den[:])
                sim = wpool.tile([128, S], dt.float32, tag="sim", name="sim")
                nc.vector.tensor_tensor(out=sim[:], in0=dots[:], in1=rden[:], op=mybir.AluOpType.mult)
                pr = wpool.tile([128, 1], dt.float32, tag="pr", name="pr")
                nc.vector.tensor_reduce(
                    out=pr[:], in_=sim[:], axis=mybir.AxisListType.X, op=mybir.AluOpType.add,
                )
                nc.sync.dma_start(pred[t], pr[:].rearrange("p o -> (p o)"))

    nc.compile()
    return nc


_L1_TAB = None
_L2_TAB = None


def _chunk_tab(sh, pad, ch_tiles, ch_rows, ch_r0, ch_base):
    # local row (0..sh-1) -> offset within the gathered layout for rank 0;
    # final position = base_c + k*rows_c + (gl - r0_c)
    gl = np.arange(sh)
    tau = gl // 128
    tile_start = np.cumsum([0] + ch_tiles[:-1])
    c = np.searchsorted(tile_start, tau, side="right") - 1
    off = gl - np.asarray(ch_r0)[c]
    return np.asarray(ch_base)[c], np.asarray(ch_rows)[c], off


def _pad_map_l1(g):
    global _L1_TAB
    if _L1_TAB is None:
        _L1_TAB = _chunk_tab(L1_SH, L1_PAD, L1_CHUNKS, L1_CH_ROWS, L1_CH_R0, L1_CH_BASE)
    k = g // L1_SH
    gl = g % L1_SH
    base, rows, off = _L1_TAB
    return base[gl] + k * rows[gl] + off[gl]


def _pad_map_l2(q):
    global _L2_TAB
    if _L2_TAB is None:
        _L2_TAB = _chunk_tab(L2_SH, L2_PAD, L2_CHUNKS, L2_CH_ROWS, L2_CH_R0, L2_CH_BASE)
    k = q // L2_SH
    gl = q % L2_SH
    base, rows, off = _L2_TAB
    return base[gl] + k * rows[gl] + off[gl]


def _tileize(a, ncols):
    """[T*128 rows, ncols] -> [128, T*ncols] partition-major tile layout."""
    T = a.shape[0] // 128
    return np.ascontiguousarray(
        a.reshape(T, 128, ncols).transpose(1, 0, 2).reshape(128, T * ncols)
    ).astype(np.int32)


def kernel(features, Wself1, Wnbr1, b1, Wself2, Wnbr2, b2,
           input_nodes, nbr1, nbr2, inverse_all, source, item_rep_idx,
           n_items, n_masked):
    global _compiled
    if _compiled is None:
        _compiled = _build()
    nc = _compiled

    features = np.asarray(features, dtype=np.float32)
    input_nodes = np.asarray(input_nodes, dtype=np.int64)
    nbr1 = np.asarray(nbr1, dtype=np.int64)
    nbr2 = np.asarray(nbr2, dtype=np.int64)
    inverse_all = np.asarray(inverse_all, dtype=np.int64)
    source = np.asarray(source, dtype=np.int64)
    item_rep_idx = np.asarray(item_rep_idx, dtype=np.int64)

    scale = np.float32(1.0 / S)
    w1n_s = (np.asarray(Wnbr1, np.float32) * scale).astype(np.float32)
    w2n_s = (np.asarray(Wnbr2, np.float32) * scale).astype(np.float32)

    common = {
        "feat": features,
        "w1s": np.asarray(Wself1, np.float32),
        "w1n": w1n_s,
        "b1": np.tile(np.asarray(b1, np.float32).reshape(1, HID), (128, 1)),
        "w2s": np.asarray(Wself2, np.float32),
        "w2n": w2n_s,
        "b2": np.tile(np.asarray(b2, np.float32).reshape(1, OUT), (128, 1)),
    }

    in_maps = []
    for k in range(NCORES):
        # ---- layer 1 indices (into features) ----
        d0 = k * L1_SH
        d = np.arange(L1_PAD) + d0
        real = d < d0 + L1_SH
        d_c = np.where(real, d, d0)  # clamp padding to a real row
        l1n_idx = input_nodes[nbr1[d_c]]           # [L1_PAD, S]
        l1s_idx = input_nodes[d_c][:, None]        # [L1_PAD, 1]

        # ---- layer 2 indices (into padded h1) ----
        g0 = k * L2_SH
        g = np.arange(L2_PAD) + g0
        realg = g < g0 + L2_SH
        g_c = np.where(realg, g, g0)
        l2n_idx = _pad_map_l1(nbr1_safe := nbr2[g_c])  # [L2_PAD, S]
        l2s_idx = _pad_map_l1(g_c)[:, None]

        # ---- head indices (into padded h2) ----
        u0 = k * HD_USERS
        u = np.arange(HD_USERS) + u0
        src = source.reshape(B, S)[u]               # [512, S]
        masked = src < n_masked
        a_q = n_items + (src - n_masked)            # all_emb row if unmasked
        a_idx = np.where(masked, OOB, _pad_map_l2(inverse_all[np.where(masked, 0, a_q)]))
        b_idx = _pad_map_l2(inverse_all[item_rep_idx.reshape(B, S)[u, 0]])[:, None]

        in_maps.append({
            **common,
            "l1n": _tileize(l1n_idx, S),
            "l1s": _tileize(l1s_idx, 1),
            "l2n": _tileize(l2n_idx, S),
            "l2s": _tileize(l2s_idx, 1),
            "hda": _tileize(a_idx, S),
            "hdb": _tileize(b_idx, 1),
        })

    res = run_bass_kernel_spmd(nc, in_maps, core_ids=list(range(NCORES)))
    pred = np.concatenate(
        [res.results[k]["pred"].reshape(HD_T, 128).reshape(-1) for k in range(NCORES)]
    )
    return pred.astype(np.float32)


# revision 7
# speedup vs baseline: 1.4466x; 1.0011x over previous
"""GraphSAGE predictor on 8 Trainium2 NeuronCores (Bass/Tile).

Strategy (data-parallel over dst nodes, hint-aligned):
- Layer 1: each core computes 12500 of the 100000 dst rows. Feature rows are
  gathered with indirect DMA; the 10-neighbor sum is folded into the gather
  via the SDMA CCE add (compute_op=add into a memset tile). Self/neighbor
  terms go through PE transposes + matmuls, bias+relu fused on DVE/ACT.
- AllGather h1 across the 8 cores (padded shards of 12544 rows).
- Layer 2: same pattern over 5512 dst rows/core reading the gathered h1.
- AllGather h2, then the cosine-similarity head: per user, the item row is
  constant across the 10 sources, so b is gathered once per user; masked
  sources use an out-of-bounds sentinel + bounds_check so memset zeros
  survive, reproducing the zero-padding of rated_emb.
- All index composition (input_nodes[nbr1], padded-shard remapping,
  inverse_all composition) is host-side numpy on int32 index arrays only.
"""
import numpy as np

import concourse.bass as bass
import concourse.bacc as bacc
import concourse.tile as tile
import concourse.mybir as mybir
from concourse.bass_utils import run_bass_kernel_spmd
from concourse.masks import make_identity

NCORES = 8
N_ENT = 200000
F = 128
HID = 128
OUT = 64
N0 = 160000
N1 = 100000
N_ITEMS = 4096
N_RATED = 40000
N2 = N_ITEMS + N_RATED  # 44096
S = 10
B = 4096
R_TOT = 40960
N_MASKED = 960

L1_SH = N1 // NCORES          # 12500
L1_T = (L1_SH + 127) // 128   # 98
L1_PAD = L1_T * 128           # 12544
H1_FULL = L1_PAD * NCORES     # 100352
L1_CHUNKS = [30, 30, 30, 8]               # tiles per AllGather chunk (small tail)
L1_CH_ROWS = [c * 128 for c in L1_CHUNKS]
L1_CH_R0 = [0, 3840, 7680, 11520]         # local row start of each chunk
L1_CH_BASE = [0, 30720, 61440, 92160]     # chunk base in gathered h1

L2_SH = N2 // NCORES          # 5512
L2_T = (L2_SH + 127) // 128   # 44
L2_PAD = L2_T * 128           # 5632
H2_FULL = L2_PAD * NCORES     # 45056
L2_CHUNKS = [30, 14]
L2_CH_ROWS = [c * 128 for c in L2_CHUNKS]
L2_CH_R0 = [0, 3840]
L2_CH_BASE = [0, 30720]

HD_USERS = B // NCORES        # 512
HD_T = HD_USERS // 128        # 4

OOB = np.int32(1 << 20)

_compiled = None


def _build():
    dt = mybir.dt
    nc = bacc.Bacc("TRN2", target_bir_lowering=False, debug=False,
                   num_devices=NCORES)

    feat = nc.dram_tensor("feat", [N_ENT, F], dt.float32, kind="ExternalInput").ap()
    w1s = nc.dram_tensor("w1s", [F, HID], dt.float32, kind="ExternalInput").ap()
    w1n = nc.dram_tensor("w1n", [F, HID], dt.float32, kind="ExternalInput").ap()
    b1 = nc.dram_tensor("b1", [128, HID], dt.float32, kind="ExternalInput").ap()
    w2s = nc.dram_tensor("w2s", [HID, OUT], dt.float32, kind="ExternalInput").ap()
    w2n = nc.dram_tensor("w2n", [HID, OUT], dt.float32, kind="ExternalInput").ap()
    b2 = nc.dram_tensor("b2", [128, OUT], dt.float32, kind="ExternalInput").ap()

    l1n = nc.dram_tensor("l1n", [128, L1_T * S], dt.int32, kind="ExternalInput").ap()
    l1s = nc.dram_tensor("l1s", [128, L1_T], dt.int32, kind="ExternalInput").ap()
    l2n = nc.dram_tensor("l2n", [128, L2_T * S], dt.int32, kind="ExternalInput").ap()
    l2s = nc.dram_tensor("l2s", [128, L2_T], dt.int32, kind="ExternalInput").ap()
    hda = nc.dram_tensor("hda", [128, HD_T * S], dt.int32, kind="ExternalInput").ap()
    hdb = nc.dram_tensor("hdb", [128, HD_T], dt.int32, kind="ExternalInput").ap()

    pred = nc.dram_tensor("pred", [HD_T, 128], dt.float32, kind="ExternalOutput").ap()

    h1_loc = nc.dram_tensor("h1_loc", [L1_PAD, HID], dt.float32).ap()
    h1_full = nc.dram_tensor("h1_full", [H1_FULL, HID], dt.float32).ap()
    h2_loc = nc.dram_tensor("h2_loc", [L2_PAD, OUT], dt.float32).ap()
    h2_full = nc.dram_tensor("h2_full", [H2_FULL, OUT], dt.float32).ap()

    groups = [list(range(NCORES))]

    with tile.TileContext(nc) as tc:
        with (
            tc.tile_pool(name="const", bufs=1) as cpool,
            tc.tile_pool(name="work", bufs=6) as wpool,
            tc.tile_pool(name="psum", bufs=3, space="PSUM") as ppool,
            tc.tile_pool(name="psumo", bufs=2, space="PSUM") as ppool_o,
        ):
            ident = cpool.tile([128, 128], dt.float32)
            make_identity(nc, ident[:])
            w1s_t = cpool.tile([F, HID], dt.float32)
            nc.sync.dma_start(w1s_t[:], w1s)
            w1n_t = cpool.tile([F, HID], dt.float32)
            nc.sync.dma_start(w1n_t[:], w1n)
            b1_t = cpool.tile([128, HID], dt.float32)
            nc.sync.dma_start(b1_t[:], b1)
            w2s_t = cpool.tile([HID, OUT], dt.float32)
            nc.sync.dma_start(w2s_t[:], w2s)
            w2n_t = cpool.tile([HID, OUT], dt.float32)
            nc.sync.dma_start(w2n_t[:], w2n)
            b2_t = cpool.tile([128, OUT], dt.float32)
            nc.sync.dma_start(b2_t[:], b2)

            l1n_t = cpool.tile([128, L1_T * S], dt.int32)
            nc.sync.dma_start(l1n_t[:], l1n)
            l1s_t = cpool.tile([128, L1_T], dt.int32)
            nc.sync.dma_start(l1s_t[:], l1s)
            l2n_t = cpool.tile([128, L2_T * S], dt.int32)
            nc.sync.dma_start(l2n_t[:], l2n)
            l2s_t = cpool.tile([128, L2_T], dt.int32)
            nc.sync.dma_start(l2s_t[:], l2s)
            hda_t = cpool.tile([128, HD_T * S], dt.int32)
            nc.sync.dma_start(hda_t[:], hda)
            hdb_t = cpool.tile([128, HD_T], dt.int32)
            nc.sync.dma_start(hdb_t[:], hdb)

            h1v = h1_loc.rearrange("(t p) f -> t p f", p=128)
            h2v = h2_loc.rearrange("(t p) f -> t p f", p=128)

            def sage_tile(t, nidx, sidx, src, wself, wnbr, bias_t, d_out, relu,
                          out_view):
                wide = wpool.tile([128, S * F], dt.float32, tag="wide", name="wide")
                for j in range(S):
                    c = t * S + j
                    nc.gpsimd.indirect_dma_start(
                        out=wide[:, j * F:(j + 1) * F], out_offset=None, in_=src,
                        in_offset=bass.IndirectOffsetOnAxis(ap=nidx[:, c:c + 1], axis=0),
                    )
                hself = wpool.tile([128, F], dt.float32, tag="hself", name="hself")
                nc.gpsimd.indirect_dma_start(
                    out=hself[:], out_offset=None, in_=src,
                    in_offset=bass.IndirectOffsetOnAxis(ap=sidx[:, t:t + 1], axis=0),
                )
                pnb = ppool.tile([128, 128], dt.float32, tag="pnb", name="pnb")
                for j in range(S):
                    nc.tensor.matmul(pnb[:], lhsT=wide[:, j * F:(j + 1) * F],
                                     rhs=ident[:], is_transpose=True,
                                     start=(j == 0), stop=(j == S - 1))
                psf = ppool.tile([128, 128], dt.float32, tag="psf", name="psf")
                nc.tensor.transpose(psf[:], hself[:], ident[:])
                nbT = wpool.tile([128, 128], dt.float32, tag="nbT", name="nbT")
                nc.vector.tensor_copy(nbT[:], pnb[:])
                sfT = wpool.tile([128, 128], dt.float32, tag="sfT", name="sfT")
                nc.vector.tensor_copy(sfT[:], psf[:])
                pout = ppool_o.tile([128, d_out], dt.float32, tag="pout", name="pout")
                nc.tensor.matmul(pout[:], lhsT=sfT[:], rhs=wself[:], start=True, stop=False)
                nc.tensor.matmul(pout[:], lhsT=nbT[:], rhs=wnbr[:], start=False, stop=True)
                hout = wpool.tile([128, d_out], dt.float32, tag="hout", name="hout")
                nc.vector.tensor_tensor(
                    out=hout[:], in0=pout[:], in1=bias_t[:, :d_out],
                    op=mybir.AluOpType.add,
                )
                if relu:
                    nc.scalar.activation(hout[:], hout[:], mybir.ActivationFunctionType.Relu)
                nc.sync.dma_start(out_view[t], hout[:])

            t = 0
            for c, ntiles in enumerate(L1_CHUNKS):
                for _ in range(ntiles):
                    sage_tile(t, l1n_t, l1s_t, feat, w1s_t, w1n_t, b1_t, HID, True, h1v)
                    t += 1
                r0, rows, base = L1_CH_R0[c], L1_CH_ROWS[c], L1_CH_BASE[c]
                nc.gpsimd.collective_compute(
                    "AllGather", mybir.AluOpType.bypass, replica_groups=groups,
                    ins=[h1_loc[r0:r0 + rows].opt()],
                    outs=[h1_full[base:base + NCORES * rows].opt()],
                )

            t = 0
            for c, ntiles in enumerate(L2_CHUNKS):
                for _ in range(ntiles):
                    sage_tile(t, l2n_t, l2s_t, h1_full, w2s_t, w2n_t, b2_t, OUT, False, h2v)
                    t += 1
                r0, rows, base = L2_CH_R0[c], L2_CH_ROWS[c], L2_CH_BASE[c]
                nc.gpsimd.collective_compute(
                    "AllGather", mybir.AluOpType.bypass, replica_groups=groups,
                    ins=[h2_loc[r0:r0 + rows].opt()],
                    outs=[h2_full[base:base + NCORES * rows].opt()],
                )

            for t in range(HD_T):
                at = wpool.tile([128, S * OUT], dt.float32, tag="at", name="at")
                nc.vector.memset(at[:], 0.0)
                for j in range(S):
                    c = t * S + j
                    nc.gpsimd.indirect_dma_start(
                        out=at[:, j * OUT:(j + 1) * OUT], out_offset=None, in_=h2_full,
                        in_offset=bass.IndirectOffsetOnAxis(ap=hda_t[:, c:c + 1], axis=0),
                        bounds_check=H2_FULL - 1, oob_is_err=False,
                    )
                bt = wpool.tile([128, OUT], dt.float32, tag="bt", name="bt")
                nc.gpsimd.indirect_dma_start(
                    out=bt[:], out_offset=None, in_=h2_full,
                    in_offset=bass.IndirectOffsetOnAxis(ap=hdb_t[:, t:t + 1], axis=0),
                )
                ab = wpool.tile([128, S * OUT], dt.float32, tag="ab", name="ab")
                nc.vector.tensor_tensor(
                    out=ab[:].rearrange("p (s d) -> p s d", d=OUT),
                    in0=at[:].rearrange("p (s d) -> p s d", d=OUT),
                    in1=bt[:].unsqueeze(1).to_broadcast([128, S, OUT]),
                    op=mybir.AluOpType.mult,
                )
                dots = wpool.tile([128, S], dt.float32, tag="dots", name="dots")
                nc.vector.tensor_reduce(
                    out=dots[:], in_=ab[:].rearrange("p (s d) -> p s d", d=OUT),
                    axis=mybir.AxisListType.X, op=mybir.AluOpType.add,
                )
                nc.vector.tensor_tensor(out=ab[:], in0=at[:], in1=at[:], op=mybir.AluOpType.mult)
                na2 = wpool.tile([128, S], dt.float32, tag="na2", name="na2")
                nc.vector.tensor_reduce(
                    out=na2[:], in_=ab[:].rearrange("p (s d) -> p s d", d=OUT),
                    axis=mybir.AxisListType.X, op=mybir.AluOpType.add,
                )
                bb = wpool.tile([128, OUT], dt.float32, tag="bb", name="bb")
                nc.vector.tensor_tensor(out=bb[:], in0=bt[:], in1=bt[:], op=mybir.AluOpType.mult)
                nb2 = wpool.tile([128, 1], dt.float32, tag="nb2", name="nb2")
                nc.vector.tensor_reduce(
                    out=nb2[:], in_=bb[:], axis=mybir.AxisListType.X, op=mybir.AluOpType.add,
                )
                na = wpool.tile([128, S], dt.float32, tag="na", name="na")
                nc.scalar.activation(na[:], na2[:], mybir.ActivationFunctionType.Sqrt)
                nc.vector.tensor_scalar_max(na[:], na[:], 1e-6)
                nb = wpool.tile([128, 1], dt.float32, tag="nb", name="nb")
                nc.scalar.activation(nb[:], nb2[:], mybir.ActivationFunctionType.Sqrt)
                nc.vector.tensor_scalar_max(nb[:], nb[:], 1e-6)
                den = wpool.tile([128, S], dt.float32, tag="den", name="den")
                nc.vector.tensor_tensor(
                    out=den[:], in0=na[:], in1=nb[:].to_broadcast([128, S]),
                    op=mybir.AluOpType.mult,
                )
                rden = wpool.tile([128, S], dt.float32, tag="rden", name="rden")
                nc.vector.reciprocal(rden[:], den[:])
                sim = wpool.tile([128, S], dt.float32, tag="sim", name="sim")
                nc.vector.tensor_tensor(out=sim[:], in0=dots[:], in1=rden[:], op=mybir.AluOpType.mult)
                pr = wpool.tile([128, 1], dt.float32, tag="pr", name="pr")
                nc.vector.tensor_reduce(
                    out=pr[:], in_=sim[:], axis=mybir.AxisListType.X, op=mybir.AluOpType.add,
                )
                nc.sync.dma_start(pred[t], pr[:].rearrange("p o -> (p o)"))

    nc.compile()
    return nc


_L1_TAB = None
_L2_TAB = None


def _chunk_tab(sh, pad, ch_tiles, ch_rows, ch_r0, ch_base):
    # local row (0..sh-1) -> offset within the gathered layout for rank 0;
    # final position = base_c + k*rows_c + (gl - r0_c)
    gl = np.arange(sh)
    tau = gl // 128
    tile_start = np.cumsum([0] + ch_tiles[:-1])
    c = np.searchsorted(tile_start, tau, side="right") - 1
    off = gl - np.asarray(ch_r0)[c]
    return np.asarray(ch_base)[c], np.asarray(ch_rows)[c], off


def _pad_map_l1(g):
    global _L1_TAB
    if _L1_TAB is None:
        _L1_TAB = _chunk_tab(L1_SH, L1_PAD, L1_CHUNKS, L1_CH_ROWS, L1_CH_R0, L1_CH_BASE)
    k = g // L1_SH
    gl = g % L1_SH
    base, rows, off = _L1_TAB
    return base[gl] + k * rows[gl] + off[gl]


def _pad_map_l2(q):
    global _L2_TAB
    if _L2_TAB is None:
        _L2_TAB = _chunk_tab(L2_SH, L2_PAD, L2_CHUNKS, L2_CH_ROWS, L2_CH_R0, L2_CH_BASE)
    k = q // L2_SH
    gl = q % L2_SH
    base, rows, off = _L2_TAB
    return base[gl] + k * rows[gl] + off[gl]


def _tileize(a, ncols):
    """[T*128 rows, ncols] -> [128, T*ncols] partition-major tile layout."""
    T = a.shape[0] // 128
    return np.ascontiguousarray(
        a.reshape(T, 128, ncols).transpose(1, 0, 2).reshape(128, T * ncols)
    ).astype(np.int32)


def kernel(features, Wself1, Wnbr1, b1, Wself2, Wnbr2, b2,
           input_nodes, nbr1, nbr2, inverse_all, source, item_rep_idx,
           n_items, n_masked):
    global _compiled
    if _compiled is None:
        _compiled = _build()
    nc = _compiled

    features = np.asarray(features, dtype=np.float32)
    input_nodes = np.asarray(input_nodes, dtype=np.int64)
    nbr1 = np.asarray(nbr1, dtype=np.int64)
    nbr2 = np.asarray(nbr2, dtype=np.int64)
    inverse_all = np.asarray(inverse_all, dtype=np.int64)
    source = np.asarray(source, dtype=np.int64)
    item_rep_idx = np.asarray(item_rep_idx, dtype=np.int64)

    scale = np.float32(1.0 / S)
    w1n_s = (np.asarray(Wnbr1, np.float32) * scale).astype(np.float32)
    w2n_s = (np.asarray(Wnbr2, np.float32) * scale).astype(np.float32)

    common = {
        "feat": features,
        "w1s": np.asarray(Wself1, np.float32),
        "w1n": w1n_s,
        "b1": np.tile(np.asarray(b1, np.float32).reshape(1, HID), (128, 1)),
        "w2s": np.asarray(Wself2, np.float32),
        "w2n": w2n_s,
        "b2": np.tile(np.asarray(b2, np.float32).reshape(1, OUT), (128, 1)),
    }

    in_maps = []
    for k in range(NCORES):
        # ---- layer 1 indices (into features) ----
        d0 = k * L1_SH
        d = np.arange(L1_PAD) + d0
        real = d < d0 + L1_SH
        d_c = np.where(real, d, d0)  # clamp padding to a real row
        l1n_idx = input_nodes[nbr1[d_c]]           # [L1_PAD, S]
        l1s_idx = input_nodes[d_c][:, None]        # [L1_PAD, 1]

        # ---- layer 2 indices (into padded h1) ----
        g0 = k * L2_SH
        g = np.arange(L2_PAD) + g0
        realg = g < g0 + L2_SH
        g_c = np.where(realg, g, g0)
        l2n_idx = _pad_map_l1(nbr1_safe := nbr2[g_c])  # [L2_PAD, S]
        l2s_idx = _pad_map_l1(g_c)[:, None]

        # ---- head indices (into padded h2) ----
        u0 = k * HD_USERS
        u = np.arange(HD_USERS) + u0
        src = source.reshape(B, S)[u]               # [512, S]
        masked = src < n_masked
        a_q = n_items + (src - n_masked)            # all_emb row if unmasked
        a_idx = np.where(masked, OOB, _pad_map_l2(inverse_all[np.where(masked, 0, a_q)]))
        b_idx = _pad_map_l2(inverse_all[item_rep_idx.reshape(B, S)[u, 0]])[:, None]

        in_maps.append({
            **common,
            "l1n": _tileize(l1n_idx, S),
            "l1s": _tileize(l1s_idx, 1),
            "l2n": _tileize(l2n_idx, S),
            "l2s": _tileize(l2s_idx, 1),
            "hda": _tileize(a_idx, S),
            "hdb": _tileize(b_idx, 1),
        })

    res = run_bass_kernel_spmd(nc, in_maps, core_ids=list(range(NCORES)))
    pred = np.concatenate(
        [res.results[k]["pred"].reshape(HD_T, 128).reshape(-1) for k in range(NCORES)]
    )
    return pred.astype(np.float32)


# revision 8
# speedup vs baseline: 1.4527x; 1.0043x over previous
"""GraphSAGE predictor on 8 Trainium2 NeuronCores (Bass/Tile).

Strategy (data-parallel over dst nodes, hint-aligned):
- Layer 1: each core computes 12500 of the 100000 dst rows. Feature rows are
  gathered with indirect DMA; the 10-neighbor sum is folded into the gather
  via the SDMA CCE add (compute_op=add into a memset tile). Self/neighbor
  terms go through PE transposes + matmuls, bias+relu fused on DVE/ACT.
- AllGather h1 across the 8 cores (padded shards of 12544 rows).
- Layer 2: same pattern over 5512 dst rows/core reading the gathered h1.
- AllGather h2, then the cosine-similarity head: per user, the item row is
  constant across the 10 sources, so b is gathered once per user; masked
  sources use an out-of-bounds sentinel + bounds_check so memset zeros
  survive, reproducing the zero-padding of rated_emb.
- All index composition (input_nodes[nbr1], padded-shard remapping,
  inverse_all composition) is host-side numpy on int32 index arrays only.
"""
import numpy as np

import concourse.bass as bass
import concourse.bacc as bacc
import concourse.tile as tile
import concourse.mybir as mybir
from concourse.bass_utils import run_bass_kernel_spmd
from concourse.masks import make_identity

NCORES = 8
N_ENT = 200000
F = 128
HID = 128
OUT = 64
N0 = 160000
N1 = 100000
N_ITEMS = 4096
N_RATED = 40000
N2 = N_ITEMS + N_RATED  # 44096
S = 10
B = 4096
R_TOT = 40960
N_MASKED = 960

L1_SH = N1 // NCORES          # 12500
L1_T = (L1_SH + 127) // 128   # 98
L1_PAD = L1_T * 128           # 12544
H1_FULL = L1_PAD * NCORES     # 100352
L1_CHUNKS = [30, 30, 30, 8]               # tiles per AllGather chunk (small tail)
L1_CH_ROWS = [c * 128 for c in L1_CHUNKS]
L1_CH_R0 = [0, 3840, 7680, 11520]         # local row start of each chunk
L1_CH_BASE = [0, 30720, 61440, 92160]     # chunk base in gathered h1

L2_SH = N2 // NCORES          # 5512
L2_T = (L2_SH + 127) // 128   # 44
L2_PAD = L2_T * 128           # 5632
H2_FULL = L2_PAD * NCORES     # 45056
L2_CHUNKS = [36, 8]
L2_CH_ROWS = [c * 128 for c in L2_CHUNKS]
L2_CH_R0 = [0, 4608]
L2_CH_BASE = [0, 36864]

HD_USERS = B // NCORES        # 512
HD_T = HD_USERS // 128        # 4

OOB = np.int32(1 << 20)

_compiled = None


def _build():
    dt = mybir.dt
    nc = bacc.Bacc("TRN2", target_bir_lowering=False, debug=False,
                   num_devices=NCORES)

    feat = nc.dram_tensor("feat", [N_ENT, F], dt.float32, kind="ExternalInput").ap()
    w1s = nc.dram_tensor("w1s", [F, HID], dt.float32, kind="ExternalInput").ap()
    w1n = nc.dram_tensor("w1n", [F, HID], dt.float32, kind="ExternalInput").ap()
    b1 = nc.dram_tensor("b1", [128, HID], dt.float32, kind="ExternalInput").ap()
    w2s = nc.dram_tensor("w2s", [HID, OUT], dt.float32, kind="ExternalInput").ap()
    w2n = nc.dram_tensor("w2n", [HID, OUT], dt.float32, kind="ExternalInput").ap()
    b2 = nc.dram_tensor("b2", [128, OUT], dt.float32, kind="ExternalInput").ap()

    l1n = nc.dram_tensor("l1n", [128, L1_T * S], dt.int32, kind="ExternalInput").ap()
    l1s = nc.dram_tensor("l1s", [128, L1_T], dt.int32, kind="ExternalInput").ap()
    l2n = nc.dram_tensor("l2n", [128, L2_T * S], dt.int32, kind="ExternalInput").ap()
    l2s = nc.dram_tensor("l2s", [128, L2_T], dt.int32, kind="ExternalInput").ap()
    hda = nc.dram_tensor("hda", [128, HD_T * S], dt.int32, kind="ExternalInput").ap()
    hdb = nc.dram_tensor("hdb", [128, HD_T], dt.int32, kind="ExternalInput").ap()

    pred = nc.dram_tensor("pred", [HD_T, 128], dt.float32, kind="ExternalOutput").ap()

    h1_loc = nc.dram_tensor("h1_loc", [L1_PAD, HID], dt.float32).ap()
    h1_full = nc.dram_tensor("h1_full", [H1_FULL, HID], dt.float32).ap()
    h2_loc = nc.dram_tensor("h2_loc", [L2_PAD, OUT], dt.float32).ap()
    h2_full = nc.dram_tensor("h2_full", [H2_FULL, OUT], dt.float32).ap()

    groups = [list(range(NCORES))]

    with tile.TileContext(nc) as tc:
        with (
            tc.tile_pool(name="const", bufs=1) as cpool,
            tc.tile_pool(name="work", bufs=8) as wpool,
            tc.tile_pool(name="psum", bufs=3, space="PSUM") as ppool,
            tc.tile_pool(name="psumo", bufs=2, space="PSUM") as ppool_o,
        ):
            ident = cpool.tile([128, 128], dt.float32)
            make_identity(nc, ident[:])
            w1s_t = cpool.tile([F, HID], dt.float32)
            nc.sync.dma_start(w1s_t[:], w1s)
            w1n_t = cpool.tile([F, HID], dt.float32)
            nc.sync.dma_start(w1n_t[:], w1n)
            b1_t = cpool.tile([128, HID], dt.float32)
            nc.sync.dma_start(b1_t[:], b1)
            w2s_t = cpool.tile([HID, OUT], dt.float32)
            nc.sync.dma_start(w2s_t[:], w2s)
            w2n_t = cpool.tile([HID, OUT], dt.float32)
            nc.sync.dma_start(w2n_t[:], w2n)
            b2_t = cpool.tile([128, OUT], dt.float32)
            nc.sync.dma_start(b2_t[:], b2)

            l1n_t = cpool.tile([128, L1_T * S], dt.int32)
            nc.sync.dma_start(l1n_t[:], l1n)
            l1s_t = cpool.tile([128, L1_T], dt.int32)
            nc.sync.dma_start(l1s_t[:], l1s)
            l2n_t = cpool.tile([128, L2_T * S], dt.int32)
            nc.sync.dma_start(l2n_t[:], l2n)
            l2s_t = cpool.tile([128, L2_T], dt.int32)
            nc.sync.dma_start(l2s_t[:], l2s)
            hda_t = cpool.tile([128, HD_T * S], dt.int32)
            nc.sync.dma_start(hda_t[:], hda)
            hdb_t = cpool.tile([128, HD_T], dt.int32)
            nc.sync.dma_start(hdb_t[:], hdb)

            h1v = h1_loc.rearrange("(t p) f -> t p f", p=128)
            h2v = h2_loc.rearrange("(t p) f -> t p f", p=128)

            def sage_tile(t, nidx, sidx, src, wself, wnbr, bias_t, d_out, relu,
                          out_view):
                wide = wpool.tile([128, S * F], dt.float32, tag="wide", name="wide")
                for j in range(S):
                    c = t * S + j
                    nc.gpsimd.indirect_dma_start(
                        out=wide[:, j * F:(j + 1) * F], out_offset=None, in_=src,
                        in_offset=bass.IndirectOffsetOnAxis(ap=nidx[:, c:c + 1], axis=0),
                    )
                hself = wpool.tile([128, F], dt.float32, tag="hself", name="hself")
                nc.gpsimd.indirect_dma_start(
                    out=hself[:], out_offset=None, in_=src,
                    in_offset=bass.IndirectOffsetOnAxis(ap=sidx[:, t:t + 1], axis=0),
                )
                pnb = ppool.tile([128, 128], dt.float32, tag="pnb", name="pnb")
                for j in range(S):
                    nc.tensor.matmul(pnb[:], lhsT=wide[:, j * F:(j + 1) * F],
                                     rhs=ident[:], is_transpose=True,
                                     start=(j == 0), stop=(j == S - 1))
                psf = ppool.tile([128, 128], dt.float32, tag="psf", name="psf")
                nc.tensor.transpose(psf[:], hself[:], ident[:])
                nbT = wpool.tile([128, 128], dt.float32, tag="nbT", name="nbT")
                nc.vector.tensor_copy(nbT[:], pnb[:])
                sfT = wpool.tile([128, 128], dt.float32, tag="sfT", name="sfT")
                nc.vector.tensor_copy(sfT[:], psf[:])
                pout = ppool_o.tile([128, d_out], dt.float32, tag="pout", name="pout")
                nc.tensor.matmul(pout[:], lhsT=sfT[:], rhs=wself[:], start=True, stop=False)
                nc.tensor.matmul(pout[:], lhsT=nbT[:], rhs=wnbr[:], start=False, stop=True)
                hout = wpool.tile([128, d_out], dt.float32, tag="hout", name="hout")
                nc.vector.tensor_tensor(
                    out=hout[:], in0=pout[:], in1=bias_t[:, :d_out],
                    op=mybir.AluOpType.add,
                )
                if relu:
                    nc.scalar.activation(hout[:], hout[:], mybir.ActivationFunctionType.Relu)
                nc.sync.dma_start(out_view[t], hout[:])

            t = 0
            for c, ntiles in enumerate(L1_CHUNKS):
                for _ in range(ntiles):
                    sage_tile(t, l1n_t, l1s_t, feat, w1s_t, w1n_t, b1_t, HID, True, h1v)
                    t += 1
                r0, rows, base = L1_CH_R0[c], L1_CH_ROWS[c], L1_CH_BASE[c]
                nc.gpsimd.collective_compute(
                    "AllGather", mybir.AluOpType.bypass, replica_groups=groups,
                    ins=[h1_loc[r0:r0 + rows].opt()],
                    outs=[h1_full[base:base + NCORES * rows].opt()],
                )

            t = 0
            for c, ntiles in enumerate(L2_CHUNKS):
                for _ in range(ntiles):
                    sage_tile(t, l2n_t, l2s_t, h1_full, w2s_t, w2n_t, b2_t, OUT, False, h2v)
                    t += 1
                r0, rows, base = L2_CH_R0[c], L2_CH_ROWS[c], L2_CH_BASE[c]
                nc.gpsimd.collective_compute(
                    "AllGather", mybir.AluOpType.bypass, replica_groups=groups,
                    ins=[h2_loc[r0:r0 + rows].opt()],
                    outs=[h2_full[base:base + NCORES * rows].opt()],
                )

            for t in range(HD_T):
                at = wpool.tile([128, S * OUT], dt.float32, tag="at", name="at")
                nc.vector.memset(at[:], 0.0)
                for j in range(S):
                    c = t * S + j
                    nc.gpsimd.indirect_dma_start(
                        out=at[:, j * OUT:(j + 1) * OUT], out_offset=None, in_=h2_full,
                        in_offset=bass.IndirectOffsetOnAxis(ap=hda_t[:, c:c + 1], axis=0),
                        bounds_check=H2_FULL - 1, oob_is_err=False,
                    )
                bt = wpool.tile([128, OUT], dt.float32, tag="bt", name="bt")
                nc.gpsimd.indirect_dma_start(
                    out=bt[:], out_offset=None, in_=h2_full,
                    in_offset=bass.IndirectOffsetOnAxis(ap=hdb_t[:, t:t + 1], axis=0),
                )
                ab = wpool.tile([128, S * OUT], dt.float32, tag="ab", name="ab")
                nc.vector.tensor_tensor(
                    out=ab[:].rearrange("p (s d) -> p s d", d=OUT),
                    in0=at[:].rearrange("p (s d) -> p s d", d=OUT),
                    in1=bt[:].unsqueeze(1).to_broadcast([128, S, OUT]),
                    op=mybir.AluOpType.mult,
                )
                dots = wpool.tile([128, S], dt.float32, tag="dots", name="dots")
                nc.vector.tensor_reduce(
                    out=dots[:], in_=ab[:].rearrange("p (s d) -> p s d", d=OUT),
                    axis=mybir.AxisListType.X, op=mybir.AluOpType.add,
                )
                nc.vector.tensor_tensor(out=ab[:], in0=at[:], in1=at[:], op=mybir.AluOpType.mult)
                na2 = wpool.tile([128, S], dt.float32, tag="na2", name="na2")
                nc.vector.tensor_reduce(
                    out=na2[:], in_=ab[:].rearrange("p (s d) -> p s d", d=OUT),
                    axis=mybir.AxisListType.X, op=mybir.AluOpType.add,
                )
                bb = wpool.tile([128, OUT], dt.float32, tag="bb", name="bb")
                nc.vector.tensor_tensor(out=bb[:], in0=bt[:], in1=bt[:], op=mybir.AluOpType.mult)
                nb2 = wpool.tile([128, 1], dt.float32, tag="nb2", name="nb2")
                nc.vector.tensor_reduce(
                    out=nb2[:], in_=bb[:], axis=mybir.AxisListType.X, op=mybir.AluOpType.add,
                )
                na = wpool.tile([128, S], dt.float32, tag="na", name="na")
                nc.scalar.activation(na[:], na2[:], mybir.ActivationFunctionType.Sqrt)
                nc.vector.tensor_scalar_max(na[:], na[:], 1e-6)
                nb = wpool.tile([128, 1], dt.float32, tag="nb", name="nb")
                nc.scalar.activation(nb[:], nb2[:], mybir.ActivationFunctionType.Sqrt)
                nc.vector.tensor_scalar_max(nb[:], nb[:], 1e-6)
                den = wpool.tile([128, S], dt.float32, tag="den", name="den")
                nc.vector.tensor_tensor(
                    out=den[:], in0=na[:], in1=nb[:].to_broadcast([128, S]),
                    op=mybir.AluOpType.mult,
                )
                rden = wpool.tile([128, S], dt.float32, tag="rden", name="rden")
                nc.vector.reciprocal(rden[:], den[:])
                sim = wpool.tile([128, S], dt.float32, tag="sim", name="sim")
                nc.vector.tensor_tensor(out=sim[:], in0=dots[:], in1=rden[:], op=mybir.AluOpType.mult)
                pr = wpool.tile([128, 1], dt.float32, tag="pr", name="pr")
                nc.vector.tensor_reduce(
                    out=pr[:], in_=sim[:], axis=mybir.AxisListType.X, op=mybir.AluOpType.add,
                )
                nc.sync.dma_start(pred[t], pr[:].rearrange("p o -> (p o)"))

    nc.compile()
    return nc


_L1_TAB = None
_L2_TAB = None


def _chunk_tab(sh, pad, ch_tiles, ch_rows, ch_r0, ch_base):
    # local row (0..sh-1) -> offset within the gathered layout for rank 0;
    # final position = base_c + k*rows_c + (gl - r0_c)
    gl = np.arange(sh)
    tau = gl // 128
    tile_start = np.cumsum([0] + ch_tiles[:-1])
    c = np.searchsorted(tile_start, tau, side="right") - 1
    off = gl - np.asarray(ch_r0)[c]
    return np.asarray(ch_base)[c], np.asarray(ch_rows)[c], off


def _pad_map_l1(g):
    global _L1_TAB
    if _L1_TAB is None:
        _L1_TAB = _chunk_tab(L1_SH, L1_PAD, L1_CHUNKS, L1_CH_ROWS, L1_CH_R0, L1_CH_BASE)
    k = g // L1_SH
    gl = g % L1_SH
    base, rows, off = _L1_TAB
    return base[gl] + k * rows[gl] + off[gl]


def _pad_map_l2(q):
    global _L2_TAB
    if _L2_TAB is None:
        _L2_TAB = _chunk_tab(L2_SH, L2_PAD, L2_CHUNKS, L2_CH_ROWS, L2_CH_R0, L2_CH_BASE)
    k = q // L2_SH
    gl = q % L2_SH
    base, rows, off = _L2_TAB
    return base[gl] + k * rows[gl] + off[gl]


def _tileize(a, ncols):
    """[T*128 rows, ncols] -> [128, T*ncols] partition-major tile layout."""
    T = a.shape[0] // 128
    return np.ascontiguousarray(
        a.reshape(T, 128, ncols).transpose(1, 0, 2).reshape(128, T * ncols)
    ).astype(np.int32)


def kernel(features, Wself1, Wnbr1, b1, Wself2, Wnbr2, b2,
           input_nodes, nbr1, nbr2, inverse_all, source, item_rep_idx,
           n_items, n_masked):
    global _compiled
    if _compiled is None:
        _compiled = _build()
    nc = _compiled

    features = np.asarray(features, dtype=np.float32)
    input_nodes = np.asarray(input_nodes, dtype=np.int64)
    nbr1 = np.asarray(nbr1, dtype=np.int64)
    nbr2 = np.asarray(nbr2, dtype=np.int64)
    inverse_all = np.asarray(inverse_all, dtype=np.int64)
    source = np.asarray(source, dtype=np.int64)
    item_rep_idx = np.asarray(item_rep_idx, dtype=np.int64)

    scale = np.float32(1.0 / S)
    w1n_s = (np.asarray(Wnbr1, np.float32) * scale).astype(np.float32)
    w2n_s = (np.asarray(Wnbr2, np.float32) * scale).astype(np.float32)

    common = {
        "feat": features,
        "w1s": np.asarray(Wself1, np.float32),
        "w1n": w1n_s,
        "b1": np.tile(np.asarray(b1, np.float32).reshape(1, HID), (128, 1)),
        "w2s": np.asarray(Wself2, np.float32),
        "w2n": w2n_s,
        "b2": np.tile(np.asarray(b2, np.float32).reshape(1, OUT), (128, 1)),
    }

    in_maps = []
    for k in range(NCORES):
        # ---- layer 1 indices (into features) ----
        d0 = k * L1_SH
        d = np.arange(L1_PAD) + d0
        real = d < d0 + L1_SH
        d_c = np.where(real, d, d0)  # clamp padding to a real row
        l1n_idx = input_nodes[nbr1[d_c]]           # [L1_PAD, S]
        l1s_idx = input_nodes[d_c][:, None]        # [L1_PAD, 1]

        # ---- layer 2 indices (into padded h1) ----
        g0 = k * L2_SH
        g = np.arange(L2_PAD) + g0
        realg = g < g0 + L2_SH
        g_c = np.where(realg, g, g0)
        l2n_idx = _pad_map_l1(nbr1_safe := nbr2[g_c])  # [L2_PAD, S]
        l2s_idx = _pad_map_l1(g_c)[:, None]

        # ---- head indices (into padded h2) ----
        u0 = k * HD_USERS
        u = np.arange(HD_USERS) + u0
        src = source.reshape(B, S)[u]               # [512, S]
        masked = src < n_masked
        a_q = n_items + (src - n_masked)            # all_emb row if unmasked
        a_idx = np.where(masked, OOB, _pad_map_l2(inverse_all[np.where(masked, 0, a_q)]))
        b_idx = _pad_map_l2(inverse_all[item_rep_idx.reshape(B, S)[u, 0]])[:, None]

        in_maps.append({
            **common,
            "l1n": _tileize(l1n_idx, S),
            "l1s": _tileize(l1s_idx, 1),
            "l2n": _tileize(l2n_idx, S),
            "l2s": _tileize(l2s_idx, 1),
            "hda": _tileize(a_idx, S),
            "hdb": _tileize(b_idx, 1),
        })

    res = run_bass_kernel_spmd(nc, in_maps, core_ids=list(range(NCORES)))
    pred = np.concatenate(
        [res.results[k]["pred"].reshape(HD_T, 128).reshape(-1) for k in range(NCORES)]
    )
    return pred.astype(np.float32)
